# revision 11
# baseline (speedup 1.0000x reference)
"""Pointer-generator decode step on 8 Trainium2 NeuronCores.

Distribution (tensor parallel, 8 cores):
  - W_out (V x H) row-sharded by vocab: each core owns a 6256-slot slice
    (padded to 6272 = 49*128), host-pretransposed to [H, 6272] and streamed
    in 8 vocab-pieces so the PE matvec pipelines with the DMA.
  - type_hiddens sharded by T (512 rows/core, host-pretransposed).
  - LSTM weights sharded by hidden position (64-wide h/c slice per core);
    W_attn column-sharded (partial attention projection per core).
  - 3 tiny AllGathers: AG1 (h,c,partial proj,partial gen), AG2 (attention
    softmax stats + unnormalized exps), AG3 (vocab softmax stats).
  - scatter-add of copy attention into the vocab slice: indirect-DMA gather
    of the raw exp values straight out of the AG2 output buffer, a
    host-provided core-membership matmul applies the per-core softmax
    renormalization (and zeroes padding lanes), a host-provided selection
    matrix matmul sums duplicate targets, and an indirect-DMA scatter writes
    the per-target sums into the slice buffer (duplicates write identical
    values; pads write zeros to a dead slot).

All small per-core constants travel in one packed f32 blob (one DMA) plus
one int32 blob, routed on the ACT HWDGE ring so they never queue behind the
bulk W_out stream on the SP ring.
"""

import numpy as np

from concourse import bacc, bass, mybir
import concourse.tile as tile
from concourse.bass import IndirectOffsetOnAxis
from concourse.bass_utils import run_bass_kernel_spmd

NCORES = 8
V, OOV, H, E, T = 50000, 20, 512, 512, 4096
VO = V + OOV
SLICE = 6256
SLICE_PAD = 6272          # 49*128
NT = SLICE_PAD // 128     # 49
TSL = T // NCORES         # 512
HSL = H // NCORES         # 64
DEAD = SLICE + 4          # dead local slot in the [SLICE, SLICE_PAD) scratch range
NEG = -1.0e30
F32 = mybir.dt.float32
I32 = mybir.dt.int32
W4_PIECES = 8
W4_COLS = SLICE_PAD // W4_PIECES  # 784

# fblob column layout
C_WIH = 0
C_WHH = 1024
C_ID = 2048
C_BOUT = 2176
C_XT = 2225
C_H0T = 2229
C_BIH = 2233
C_BHH = 2237
C_WG4 = 2241
C_C0 = 2245
C_NH = 2246
C_TH = 2247
C_XS = 2248
C_WAT = 2249
C_BATN = 2761
C_BGEN = 3273
C_S = 3274                # S matrices: 128 cols per scatter tile
# C matrices (core membership): 128 cols per tile, after the S block

LAST_EXEC_TIME_NS = None
_CACHE = {}


def _fblob_width(jt):
    return C_S + 128 * jt + 128 * jt


def _build_program(jt):
    import os
    reps = int(os.environ.get("KERNEL_REPS", "1"))
    nc = bacc.Bacc(None, target_bir_lowering=False, debug=False, num_devices=NCORES)
    grp = [list(range(NCORES))]
    fw = _fblob_width(jt)
    c_cm = C_S + 128 * jt

    w4_d = nc.dram_tensor("w4", [H, SLICE_PAD], F32, kind="ExternalInput")
    tht_d = nc.dram_tensor("tht", [H, TSL], F32, kind="ExternalInput")
    fb_d = nc.dram_tensor("fb", [128, fw], F32, kind="ExternalInput")
    ib_d = nc.dram_tensor("ib", [128, 2 * jt], I32, kind="ExternalInput")

    out_lp = nc.dram_tensor("out_lp", [128, NT], F32, kind="ExternalOutput")
    out_h = nc.dram_tensor("out_h", [HSL], F32, kind="ExternalOutput")
    out_c = nc.dram_tensor("out_c", [HSL], F32, kind="ExternalOutput")

    with tile.TileContext(nc) as tc:
        with (
            tc.tile_pool(name="sb", bufs=1) as sb,
            tc.tile_pool(name="ps", bufs=1, space="PSUM") as ps,
            tc.tile_pool(name="dr", bufs=1, space="DRAM") as dr,
        ):
          def _emit():
            # ---------------- input loads ----------------
            fb = sb.tile([128, fw], F32, tag="fb")
            nc.scalar.dma_start(out=fb[:, 0:C_S], in_=fb_d[:, 0:C_S])
            nc.scalar.dma_start(out=fb[:, C_S:fw], in_=fb_d[:, C_S:fw])
            ib = sb.tile([128, 2 * jt], I32, tag="ib")
            nc.scalar.dma_start(out=ib[:], in_=ib_d[:])
            thtsb = sb.tile([128, 4 * TSL], F32, tag="thtsb")
            nc.scalar.dma_start(
                out=thtsb[:].rearrange("p (k f) -> p k f", k=4),
                in_=tht_d.rearrange("(k p) f -> p k f", p=128),
            )
            w4sb = sb.tile([128, 4 * SLICE_PAD], F32, tag="w4sb")
            for j in range(W4_PIECES):
                nc.sync.dma_start(
                    out=w4sb[:].rearrange("p (k f) -> p k f", k=4)[
                        :, :, j * W4_COLS:(j + 1) * W4_COLS
                    ],
                    in_=w4_d.rearrange("(k p) f -> p k f", p=128)[
                        :, :, j * W4_COLS:(j + 1) * W4_COLS
                    ],
                )
            ident = fb[:, C_ID:C_ID + 128]
            ones_c = sb.tile([128, 1], F32, tag="ones_c")
            nc.vector.memset(ones_c[:], 1.0)
            ones_r = sb.tile([1, 128], F32, tag="ones_r")
            nc.vector.memset(ones_r[:], 1.0)
            ones8 = sb.tile([8, 1], F32, tag="ones8")
            nc.vector.memset(ones8[:], 1.0)

            # ---------------- LSTM step (this core's 64-wide slice) ----------
            gates_ps = ps.tile([HSL, 4], F32, tag="early")
            for g in range(4):
                for k in range(4):
                    off = C_WIH + (g * 4 + k) * HSL
                    nc.tensor.matmul(
                        gates_ps[:, g:g + 1],
                        fb[:, off:off + HSL],
                        fb[:, C_XT + k:C_XT + k + 1],
                        start=(k == 0),
                        stop=False,
                    )
                for k in range(4):
                    off = C_WHH + (g * 4 + k) * HSL
                    nc.tensor.matmul(
                        gates_ps[:, g:g + 1],
                        fb[:, off:off + HSL],
                        fb[:, C_H0T + k:C_H0T + k + 1],
                        start=False,
                        stop=(k == 3),
                    )
            gates_sb = sb.tile([HSL, 4], F32, tag="gates")
            nc.vector.tensor_add(
                out=gates_sb[:], in0=gates_ps[:], in1=fb[0:HSL, C_BIH:C_BIH + 4]
            )
            nc.vector.tensor_add(
                out=gates_sb[:], in0=gates_sb[:], in1=fb[0:HSL, C_BHH:C_BHH + 4]
            )
            act_sb = sb.tile([HSL, 4], F32, tag="act")
            nc.scalar.activation(
                act_sb[:, 0:2], gates_sb[:, 0:2], mybir.ActivationFunctionType.Sigmoid
            )
            nc.scalar.activation(
                act_sb[:, 2:3], gates_sb[:, 2:3], mybir.ActivationFunctionType.Tanh
            )
            nc.scalar.activation(
                act_sb[:, 3:4], gates_sb[:, 3:4], mybir.ActivationFunctionType.Sigmoid
            )
            hc_sb = sb.tile([HSL, 2], F32, tag="hc")   # col0 = h, col1 = c
            tmp_sb = sb.tile([HSL, 1], F32, tag="tmp64")
            nc.vector.tensor_mul(
                out=hc_sb[:, 1:2], in0=act_sb[:, 1:2], in1=fb[0:HSL, C_C0:C_C0 + 1]
            )
            nc.vector.tensor_mul(out=tmp_sb[:], in0=act_sb[:, 0:1], in1=act_sb[:, 2:3])
            nc.vector.tensor_add(out=hc_sb[:, 1:2], in0=hc_sb[:, 1:2], in1=tmp_sb[:])
            tanhc_sb = sb.tile([HSL, 1], F32, tag="tanhc")
            nc.scalar.activation(
                tanhc_sb[:], hc_sb[:, 1:2], mybir.ActivationFunctionType.Tanh
            )
            nc.vector.tensor_mul(out=hc_sb[:, 0:1], in0=act_sb[:, 3:4], in1=tanhc_sb[:])
            nc.scalar.dma_start(out=out_h[:, None], in_=hc_sb[:, 0:1])
            nc.scalar.dma_start(out=out_c[:, None], in_=hc_sb[:, 1:2])

            # partial attention projection + partial gen logit
            proj_ps = ps.tile([128, 4], F32, tag="early")
            for m in range(4):
                nc.tensor.matmul(
                    proj_ps[:, m:m + 1],
                    fb[0:HSL, C_WAT + m * 128:C_WAT + (m + 1) * 128],
                    hc_sb[:, 0:1],
                    start=True,
                    stop=True,
                )
            proj_part = sb.tile([128, 4], F32, tag="proj_part")
            nc.scalar.activation(
                proj_part[:], proj_ps[:], mybir.ActivationFunctionType.Copy
            )
            gen_ps = ps.tile([1, 1], F32, tag="ptmp")
            pieces = [hc_sb[:, 0:1], fb[0:HSL, C_NH:C_NH + 1],
                      fb[0:HSL, C_TH:C_TH + 1], fb[0:HSL, C_XS:C_XS + 1]]
            for i, piece in enumerate(pieces):
                nc.tensor.matmul(
                    gen_ps[:],
                    fb[0:HSL, C_WG4 + i:C_WG4 + i + 1],
                    piece,
                    start=(i == 0),
                    stop=(i == 3),
                )
            gen_part = sb.tile([1, 1], F32, tag="gen_part")
            nc.scalar.activation(
                gen_part[:], gen_ps[:], mybir.ActivationFunctionType.Copy
            )

            # ---------------- AG1: (h,c) interleaved, proj partial, gen ------
            ag1i = dr.tile([641], F32, tag="ag1i")
            ag1o = dr.tile([NCORES, 641], F32, tag="ag1o")
            nc.scalar.dma_start(
                out=ag1i[0:128].rearrange("(p f) -> p f", f=2), in_=hc_sb[:]
            )
            nc.scalar.dma_start(
                out=ag1i[128:640].rearrange("(p f) -> p f", f=4), in_=proj_part[:]
            )
            nc.scalar.dma_start(out=ag1i[640:641, None], in_=gen_part[:])
            nc.gpsimd.collective_compute(
                "AllGather", mybir.AluOpType.bypass, replica_groups=grp,
                ins=[ag1i[:]], outs=[ag1o[:]],
            )
            # h to k-partition layout: h[j*128+p] at hk[p, j]
            hk = sb.tile([128, 4], F32, tag="hk")
            nc.scalar.dma_start(
                out=hk[0:64, :], in_=ag1o[0:8:2, 0:128:2].rearrange("c i -> i c")
            )
            nc.scalar.dma_start(
                out=hk[64:128, :], in_=ag1o[1:8:2, 0:128:2].rearrange("c i -> i c")
            )
            rhs8 = sb.tile([8, 513], F32, tag="rhs8")
            nc.scalar.dma_start(out=rhs8[:], in_=ag1o[:, 128:641])
            prsum_ps = ps.tile([1, 512], F32, tag="early")
            nc.tensor.matmul(prsum_ps[:], ones8[:], rhs8[:, 0:512], start=True, stop=True)
            gensum_ps = ps.tile([1, 1], F32, tag="ptmp")
            nc.tensor.matmul(gensum_ps[:], ones8[:], rhs8[:, 512:513], start=True, stop=True)
            projb = sb.tile([1, 512], F32, tag="projb")
            nc.vector.tensor_add(
                out=projb[:], in0=prsum_ps[:], in1=fb[0:1, C_BATN:C_BATN + 512]
            )
            gen_p = sb.tile([1, 1], F32, tag="gen_p")
            nc.scalar.activation(
                gen_p[:], gensum_ps[:], mybir.ActivationFunctionType.Sigmoid,
                bias=fb[0:1, C_BGEN:C_BGEN + 1],
            )
            one_minus = sb.tile([1, 1], F32, tag="one_minus")
            nc.vector.tensor_scalar(
                one_minus[:], gen_p[:], -1.0, 1.0,
                mybir.AluOpType.mult, mybir.AluOpType.add,
            )
            prsc = dr.tile([512], F32, tag="prsc")
            nc.scalar.dma_start(out=prsc[None, :], in_=projb[:, :])
            projk = sb.tile([128, 4], F32, tag="projk")
            nc.scalar.dma_start(out=projk[:], in_=prsc.rearrange("(p f) -> p f", f=4))

            # ---------------- attention scores over the T-shard --------------
            sc_ps = ps.tile([128, 4], F32, tag="early")
            for m in range(4):
                for k in range(4):
                    nc.tensor.matmul(
                        sc_ps[:, m:m + 1],
                        thtsb[:, k * TSL + m * 128:k * TSL + (m + 1) * 128],
                        projk[:, k:k + 1],
                        start=(k == 0),
                        stop=(k == 3),
                    )
            scb = sb.tile([128, 4], F32, tag="scb")
            nc.vector.tensor_copy(out=scb[:], in_=sc_ps[:])
            scmax_p = sb.tile([128, 1], F32, tag="scmax_p")
            nc.vector.reduce_max(out=scmax_p[:], in_=scb[:], axis=mybir.AxisListType.X)
            scmax_t_ps = ps.tile([1, 128], F32, tag="ptmp")
            nc.tensor.transpose(scmax_t_ps[:], scmax_p[:], ident)
            scmax_t = sb.tile([1, 128], F32, tag="scmax_t")
            nc.vector.tensor_copy(out=scmax_t[:], in_=scmax_t_ps[:])
            mtst = sb.tile([1, 2], F32, tag="mtst")
            nc.vector.reduce_max(
                out=mtst[:, 0:1], in_=scmax_t[:], axis=mybir.AxisListType.X
            )
            negmt = sb.tile([1, 1], F32, tag="negmt")
            nc.vector.tensor_scalar_mul(negmt[:], mtst[:, 0:1], -1.0)
            negmt_b_ps = ps.tile([128, 1], F32, tag="ptmp")
            nc.tensor.matmul(negmt_b_ps[:], ones_r[:], negmt[:], start=True, stop=True)
            negmt_b = sb.tile([128, 1], F32, tag="negmt_b")
            nc.vector.tensor_copy(out=negmt_b[:], in_=negmt_b_ps[:])
            u_sb = sb.tile([128, 4], F32, tag="u_sb")
            st_p = sb.tile([128, 1], F32, tag="st_p")
            nc.scalar.activation(
                u_sb[:], scb[:], mybir.ActivationFunctionType.Exp,
                bias=negmt_b[:], accum_out=st_p[:],
            )
            st_ps = ps.tile([1, 1], F32, tag="ptmp")
            nc.tensor.matmul(st_ps[:], st_p[:], ones_c[:], start=True, stop=True)
            nc.vector.tensor_copy(out=mtst[:, 1:2], in_=st_ps[:])

            # ---------------- AG2: u payload + stats --------------------------
            ag2i = dr.tile([TSL + 2], F32, tag="ag2i")
            ag2o = dr.tile([NCORES, TSL + 2], F32, tag="ag2o")
            nc.scalar.dma_start(
                out=ag2i[0:TSL].rearrange("(p f) -> p f", f=4), in_=u_sb[:]
            )
            nc.scalar.dma_start(
                out=ag2i[TSL:TSL + 2].rearrange("a -> () a"), in_=mtst[:]
            )
            nc.gpsimd.collective_compute(
                "AllGather", mybir.AluOpType.bypass, replica_groups=grp,
                ins=[ag2i[:]], outs=[ag2o[:]],
            )
            mrow_t = sb.tile([1, 8], F32, tag="mrow_t")
            nc.scalar.dma_start(
                out=mrow_t[:], in_=ag2o[:, TSL:TSL + 1].rearrange("c x -> x c")
            )
            srow_t = sb.tile([1, 8], F32, tag="srow_t")
            nc.scalar.dma_start(
                out=srow_t[:], in_=ag2o[:, TSL + 1:TSL + 2].rearrange("c x -> x c")
            )
            mtg = sb.tile([1, 1], F32, tag="mtg")
            nc.vector.reduce_max(out=mtg[:], in_=mrow_t[:], axis=mybir.AxisListType.X)
            drow = sb.tile([1, 8], F32, tag="drow")
            nc.vector.tensor_scalar(
                drow[:], mrow_t[:], mtg[:, 0:1], None, mybir.AluOpType.subtract
            )
            erow = sb.tile([1, 8], F32, tag="erow")
            nc.scalar.activation(erow[:], drow[:], mybir.ActivationFunctionType.Exp)
            frow = sb.tile([1, 8], F32, tag="frow")
            nc.vector.tensor_mul(out=frow[:], in0=erow[:], in1=srow_t[:])
            stg = sb.tile([1, 1], F32, tag="stg")
            nc.vector.reduce_sum(out=stg[:], in_=frow[:], axis=mybir.AxisListType.X)
            rcp_stg = sb.tile([1, 1], F32, tag="rcp_stg")
            nc.vector.reciprocal(rcp_stg[:], stg[:])
            cscale = sb.tile([1, 1], F32, tag="cscale")
            nc.vector.tensor_mul(out=cscale[:], in0=one_minus[:], in1=rcp_stg[:])
            scale_row = sb.tile([1, 8], F32, tag="scale_row")
            nc.vector.tensor_scalar(
                scale_row[:], erow[:], cscale[:, 0:1], None, mybir.AluOpType.mult
            )
            scale_t_ps = ps.tile([8, 1], F32, tag="ptmp")
            nc.tensor.transpose(scale_t_ps[:], scale_row[:], ident[0:1, 0:1])
            scale_t = sb.tile([8, 1], F32, tag="scale_t")
            nc.vector.tensor_copy(out=scale_t[:], in_=scale_t_ps[:])

            # ---------------- scatter into the vocab slice -------------------
            pcopy_dram = dr.tile([SLICE_PAD, 1], F32, tag="pcopy_dram")
            zero49 = sb.tile([128, NT], F32, tag="zero49")
            nc.vector.memset(zero49[:], 0.0)
            nc.scalar.dma_start(
                out=pcopy_dram[:, 0].rearrange("(p f) -> p f", f=NT), in_=zero49[:]
            )
            ag2flat = ag2o.rearrange("a b -> (a b)")
            vals_l, ssum_l = [], []
            for m in range(jt):
                vals = sb.tile([128, 1], F32, tag="sc_vals", bufs=4,
                               name=f"vals{m}")
                nc.gpsimd.indirect_dma_start(
                    out=vals[:],
                    out_offset=None,
                    in_=ag2flat[:, None],
                    in_offset=IndirectOffsetOnAxis(ap=ib[:, m:m + 1], axis=0),
                )
                vals_l.append(vals)
            for m in range(jt):
                svec_ps = ps.tile([128, 1], F32, tag="scvec", bufs=2,
                                  name=f"svec{m}")
                nc.tensor.matmul(
                    svec_ps[:],
                    fb[0:8, c_cm + m * 128:c_cm + (m + 1) * 128],
                    scale_t[:],
                    start=True, stop=True,
                )
                vs = sb.tile([128, 1], F32, tag="sc_vs", bufs=4, name=f"vs{m}")
                nc.vector.tensor_mul(out=vs[:], in0=vals_l[m][:], in1=svec_ps[:])
                ssum_ps = ps.tile([128, 1], F32, tag="scsum", bufs=2,
                                  name=f"ssps{m}")
                nc.tensor.matmul(
                    ssum_ps[:],
                    fb[:, C_S + m * 128:C_S + (m + 1) * 128],
                    vs[:],
                    start=True, stop=True,
                )
                ssum = sb.tile([128, 1], F32, tag="sc_ssum", bufs=4,
                               name=f"ssum{m}")
                nc.vector.tensor_copy(out=ssum[:], in_=ssum_ps[:])
                ssum_l.append(ssum)
            for m in range(jt):
                nc.gpsimd.indirect_dma_start(
                    out=pcopy_dram[:],
                    out_offset=IndirectOffsetOnAxis(ap=ib[:, jt + m:jt + m + 1], axis=0),
                    in_=ssum_l[m][:],
                    in_offset=None,
                )
            pc_sb = sb.tile([128, NT], F32, tag="pc_sb")
            nc.scalar.dma_start(
                out=pc_sb[:], in_=pcopy_dram[:, 0].rearrange("(p f) -> p f", f=NT)
            )

            # ---------------- vocab logits ------------------------------------
            pl = [ps.tile([128, 25], F32, tag=f"pl{i}", name=f"pl{i}") for i in range(2)]
            for t in range(NT):
                pt = pl[t // 25]
                col = t % 25
                for k in range(4):
                    nc.tensor.matmul(
                        pt[:, col:col + 1],
                        w4sb[:, k * SLICE_PAD + t * 128:k * SLICE_PAD + (t + 1) * 128],
                        hk[:, k:k + 1],
                        start=(k == 0),
                        stop=(k == 3),
                    )
            logits_sb = sb.tile([128, NT], F32, tag="logits")
            nc.vector.tensor_add(
                out=logits_sb[:, 0:25], in0=pl[0][:],
                in1=fb[:, C_BOUT:C_BOUT + 25],
            )
            nc.vector.tensor_add(
                out=logits_sb[:, 25:NT], in0=pl[1][:, 0:NT - 25],
                in1=fb[:, C_BOUT + 25:C_BOUT + NT],
            )
            lmax_p = sb.tile([128, 1], F32, tag="lmax_p")
            nc.vector.reduce_max(
                out=lmax_p[:], in_=logits_sb[:], axis=mybir.AxisListType.X
            )
            lmax_t_ps = ps.tile([1, 128], F32, tag="ptmp")
            nc.tensor.transpose(lmax_t_ps[:], lmax_p[:], ident)
            lmax_t = sb.tile([1, 128], F32, tag="lmax_t")
            nc.vector.tensor_copy(out=lmax_t[:], in_=lmax_t_ps[:])
            mvsv = sb.tile([1, 2], F32, tag="mvsv")
            nc.vector.reduce_max(
                out=mvsv[:, 0:1], in_=lmax_t[:], axis=mybir.AxisListType.X
            )
            negmv = sb.tile([1, 1], F32, tag="negmv")
            nc.vector.tensor_scalar_mul(negmv[:], mvsv[:, 0:1], -1.0)
            negmv_b_ps = ps.tile([128, 1], F32, tag="ptmp")
            nc.tensor.matmul(negmv_b_ps[:], ones_r[:], negmv[:], start=True, stop=True)
            negmv_b = sb.tile([128, 1], F32, tag="negmv_b")
            nc.vector.tensor_copy(out=negmv_b[:], in_=negmv_b_ps[:])
            ev_sb = sb.tile([128, NT], F32, tag="ev_sb")
            sv_p = sb.tile([128, 1], F32, tag="sv_p")
            nc.scalar.activation(
                ev_sb[:], logits_sb[:], mybir.ActivationFunctionType.Exp,
                bias=negmv_b[:], accum_out=sv_p[:],
            )
            sv_ps = ps.tile([1, 1], F32, tag="ptmp")
            nc.tensor.matmul(sv_ps[:], sv_p[:], ones_c[:], start=True, stop=True)
            nc.vector.tensor_copy(out=mvsv[:, 1:2], in_=sv_ps[:])

            # ---------------- AG3: vocab stats --------------------------------
            ag3i = dr.tile([2], F32, tag="ag3i")
            ag3o = dr.tile([NCORES, 2], F32, tag="ag3o")
            nc.scalar.dma_start(out=ag3i[:].rearrange("a -> () a"), in_=mvsv[:])
            nc.gpsimd.collective_compute(
                "AllGather", mybir.AluOpType.bypass, replica_groups=grp,
                ins=[ag3i[:]], outs=[ag3o[:]],
            )
            vmrow = sb.tile([1, 8], F32, tag="vmrow")
            nc.scalar.dma_start(
                out=vmrow[:], in_=ag3o[:, 0:1].rearrange("c x -> x c")
            )
            vsrow = sb.tile([1, 8], F32, tag="vsrow")
            nc.scalar.dma_start(
                out=vsrow[:], in_=ag3o[:, 1:2].rearrange("c x -> x c")
            )
            mvg = sb.tile([1, 1], F32, tag="mvg")
            nc.vector.reduce_max(out=mvg[:], in_=vmrow[:], axis=mybir.AxisListType.X)
            vdrow = sb.tile([1, 8], F32, tag="vdrow")
            nc.vector.tensor_scalar(
                vdrow[:], vmrow[:], mvg[:, 0:1], None, mybir.AluOpType.subtract
            )
            verow = sb.tile([1, 8], F32, tag="verow")
            nc.scalar.activation(verow[:], vdrow[:], mybir.ActivationFunctionType.Exp)
            vfrow = sb.tile([1, 8], F32, tag="vfrow")
            nc.vector.tensor_mul(out=vfrow[:], in0=verow[:], in1=vsrow[:])
            svg = sb.tile([1, 1], F32, tag="svg")
            nc.vector.reduce_sum(out=svg[:], in_=vfrow[:], axis=mybir.AxisListType.X)
            dml = sb.tile([1, 1], F32, tag="dml")
            nc.vector.tensor_sub(out=dml[:], in0=mvsv[:, 0:1], in1=mvg[:])
            eml = sb.tile([1, 1], F32, tag="eml")
            nc.scalar.activation(eml[:], dml[:], mybir.ActivationFunctionType.Exp)
            rcp_svg = sb.tile([1, 1], F32, tag="rcp_svg")
            nc.vector.reciprocal(rcp_svg[:], svg[:])
            sfin = sb.tile([1, 1], F32, tag="sfin")
            nc.vector.tensor_mul(out=sfin[:], in0=eml[:], in1=gen_p[:])
            nc.vector.tensor_mul(out=sfin[:], in0=sfin[:], in1=rcp_svg[:])
            sfin_b_ps = ps.tile([128, 1], F32, tag="ptmp")
            nc.tensor.matmul(sfin_b_ps[:], ones_r[:], sfin[:], start=True, stop=True)
            sfin_b = sb.tile([128, 1], F32, tag="sfin_b")
            nc.vector.tensor_copy(out=sfin_b[:], in_=sfin_b_ps[:])

            # ---------------- final mix + log ---------------------------------
            prob_sb = sb.tile([128, NT], F32, tag="prob")
            nc.vector.tensor_scalar(
                prob_sb[:], ev_sb[:], sfin_b[:, 0:1], None, mybir.AluOpType.mult
            )
            nc.vector.tensor_add(out=prob_sb[:], in0=prob_sb[:], in1=pc_sb[:])
            nc.vector.tensor_scalar_max(prob_sb[:], prob_sb[:], 1.0e-10)
            outv_sb = sb.tile([128, NT], F32, tag="outv")
            nc.scalar.activation(
                outv_sb[:], prob_sb[:], mybir.ActivationFunctionType.Ln
            )
            nc.scalar.dma_start(out=out_lp[:], in_=outv_sb[:])

          for _ in range(reps):
              _emit()

    nc.compile()
    return nc


def _pack_scatter_tables(type_indices, core):
    """Pack this core's scatter work into tiles of 128, equal targets never
    spanning tiles. Returns (addr, tgt_rows, S, C, jt):
      addr: flat index into the AG2 output [8, 514] holding the raw exp value
      tgt_rows: permuted pcopy row (slot s -> (s%128)*NT + s//128)
      S[j, j'] = 1 if same target (duplicate-sum matrix, symmetric)
      C[c, j] = 1 if entry j sources core c (pads: all-zero column)
    """
    lo = core * SLICE
    sel = np.where((type_indices >= lo) & (type_indices < lo + SLICE))[0]
    tloc = (type_indices[sel] - lo).astype(np.int64)
    order = np.argsort(tloc, kind="stable")
    sel, tloc = sel[order], tloc[order]
    tiles = []
    cur = []
    i, n = 0, len(sel)
    while i < n:
        j = i
        while j < n and tloc[j] == tloc[i]:
            j += 1
        run = j - i
        assert run <= 128
        if len(cur) + run > 128:
            tiles.append(cur)
            cur = []
        cur.extend((int(sel[k]), int(tloc[i])) for k in range(i, j))
        i = j
    tiles.append(cur)
    jt = len(tiles)
    addr = np.zeros((jt, 128), np.int32)
    tgt = np.full((jt, 128), DEAD, np.int64)
    S = np.zeros((jt, 128, 128), np.float32)
    C = np.zeros((jt, 8, 128), np.float32)
    for m, entries in enumerate(tiles):
        tg = np.full(128, -1, np.int64)  # filler never matches a real target
        for j, (gsrc, t) in enumerate(entries):
            c, tl = gsrc // TSL, gsrc % TSL
            addr[m, j] = c * (TSL + 2) + (tl % 128) * 4 + tl // 128
            tgt[m, j] = t
            tg[j] = t
            C[m, c, j] = 1.0
        S[m] = (tg[:, None] == tg[None, :]).astype(np.float32)
    tgt_rows = ((tgt % 128) * NT + tgt // 128).astype(np.int32)
    return addr, tgt_rows, S, C, jt


def _make_in_maps(inputs):
    x = np.asarray(inputs["x"], np.float32).reshape(E)
    h0 = np.asarray(inputs["h0"], np.float32).reshape(H)
    c0 = np.asarray(inputs["c0"], np.float32).reshape(H)
    name_hidden = np.asarray(inputs["name_hidden"], np.float32).reshape(H)
    type_hidden = np.asarray(inputs["type_hidden"], np.float32).reshape(H)
    type_hiddens = np.asarray(inputs["type_hiddens"], np.float32).reshape(T, H)
    type_indices = np.asarray(inputs["type_indices"]).reshape(T).astype(np.int64)
    W_ih = np.asarray(inputs["W_ih"], np.float32)
    W_hh = np.asarray(inputs["W_hh"], np.float32)
    b_ih = np.asarray(inputs["b_ih"], np.float32).reshape(4 * H)
    b_hh = np.asarray(inputs["b_hh"], np.float32).reshape(4 * H)
    W_attn = np.asarray(inputs["W_attn"], np.float32)
    b_attn = np.asarray(inputs["b_attn"], np.float32).reshape(H)
    W_gen = np.asarray(inputs["W_gen"], np.float32).reshape(3 * H + E)
    b_gen = np.asarray(inputs["b_gen"], np.float32).reshape(1)
    W_out = np.asarray(inputs["W_out"], np.float32)
    b_out = np.asarray(inputs["b_out"], np.float32).reshape(V)

    xt = np.ascontiguousarray(x.reshape(4, 128).T)
    h0t = np.ascontiguousarray(h0.reshape(4, 128).T)
    packed = [_pack_scatter_tables(type_indices, c) for c in range(NCORES)]
    jt = max(p[4] for p in packed)
    fw = _fblob_width(jt)
    c_cm = C_S + 128 * jt
    dead_row = (DEAD % 128) * NT + DEAD // 128

    in_maps = []
    for c in range(NCORES):
        lo = c * SLICE
        wsl = np.zeros((SLICE_PAD, H), np.float32)
        n_real = max(0, min(V - lo, SLICE))
        if n_real > 0:
            wsl[:n_real] = W_out[lo:lo + n_real]
        w4 = np.ascontiguousarray(wsl.T)
        bo = np.full(SLICE_PAD, NEG, np.float32)
        if n_real > 0:
            bo[:n_real] = b_out[lo:lo + n_real]
        tht = np.ascontiguousarray(type_hiddens[c * TSL:(c + 1) * TSL].T)
        hs = slice(c * HSL, (c + 1) * HSL)

        fb = np.zeros((128, fw), np.float32)
        for g in range(4):
            rows_ih = W_ih[g * H + c * HSL:g * H + (c + 1) * HSL]
            rows_hh = W_hh[g * H + c * HSL:g * H + (c + 1) * HSL]
            for k in range(4):
                fb[:, C_WIH + (g * 4 + k) * HSL:C_WIH + (g * 4 + k + 1) * HSL] = \
                    rows_ih.T[k * 128:(k + 1) * 128]
                fb[:, C_WHH + (g * 4 + k) * HSL:C_WHH + (g * 4 + k + 1) * HSL] = \
                    rows_hh.T[k * 128:(k + 1) * 128]
            fb[0:HSL, C_BIH + g] = b_ih[g * H + c * HSL:g * H + (c + 1) * HSL]
            fb[0:HSL, C_BHH + g] = b_hh[g * H + c * HSL:g * H + (c + 1) * HSL]
        fb[:, C_ID:C_ID + 128] = np.eye(128, dtype=np.float32)
        fb[:, C_BOUT:C_BOUT + NT] = bo.reshape(NT, 128).T
        fb[:, C_XT:C_XT + 4] = xt
        fb[:, C_H0T:C_H0T + 4] = h0t
        for i in range(4):
            fb[0:HSL, C_WG4 + i] = W_gen[i * H + c * HSL:i * H + (c + 1) * HSL]
        fb[0:HSL, C_C0] = c0[hs]
        fb[0:HSL, C_NH] = name_hidden[hs]
        fb[0:HSL, C_TH] = type_hidden[hs]
        fb[0:HSL, C_XS] = x[hs]
        fb[0:HSL, C_WAT:C_WAT + H] = W_attn[:, hs].T
        perm = (np.arange(H) % 4) * 128 + np.arange(H) // 4
        fb[0, C_BATN:C_BATN + H] = b_attn[perm]
        fb[0, C_BGEN] = b_gen[0]
        addr, tgt_rows, S, C, jt_c = packed[c]
        for m in range(jt_c):
            fb[:, C_S + m * 128:C_S + (m + 1) * 128] = S[m]
            fb[0:8, c_cm + m * 128:c_cm + (m + 1) * 128] = C[m]
        # unused trailing tiles: C stays zero (gathered garbage scaled to 0),
        # S zero, targets point at the dead row
        ib = np.zeros((128, 2 * jt), np.int32)
        ib[:, jt:] = dead_row
        for m in range(jt_c):
            ib[:, m] = addr[m]
            ib[:, jt + m] = tgt_rows[m]
        in_maps.append({"w4": w4, "tht": tht, "fb": fb, "ib": ib})
    return in_maps, jt


def kernel(**inputs):
    global LAST_EXEC_TIME_NS
    import os

    in_maps, jt = _make_in_maps(inputs)
    key = ("nc", jt, os.environ.get("KERNEL_REPS", "1"))
    if key not in _CACHE:
        _CACHE[key] = _build_program(jt)
    nc = _CACHE[key]
    trace = bool(int(os.environ.get("KERNEL_TRACE", "0")))
    res = run_bass_kernel_spmd(
        nc, in_maps, core_ids=list(range(NCORES)), trace=trace,
        trace_cores=list(range(NCORES)) if trace else None,
    )
    LAST_EXEC_TIME_NS = res.exec_time_ns
    outs = res.results
    lp = np.concatenate(
        [outs[c]["out_lp"].T.ravel()[:SLICE] for c in range(NCORES)]
    )[:VO]
    h = np.concatenate([outs[c]["out_h"] for c in range(NCORES)])
    cst = np.concatenate([outs[c]["out_c"] for c in range(NCORES)])
    return (
        lp.reshape(1, VO).astype(np.float32),
        h.reshape(1, 1, H).astype(np.float32),
        cst.reshape(1, 1, H).astype(np.float32),
    )
